# revision 1
# baseline (speedup 1.0000x reference)
"""Trainium2 Bass kernel for GCNNetwork (GENConv message passing, L=6).

Strategy (graph-data parallel over 8 NeuronCores):
 - Nodes sharded contiguously: core c owns rows [7552c, 7552c+7552) (N padded
   60000 -> 60416). Edges assigned to the core owning their dst, sorted by dst.
 - Per layer: LayerNorm on the local shard -> AllGather y across cores (y_full)
   -> per 128-edge tile: indirect-DMA gather y[src], compute msg/w=exp(msg)/
   msg*w, segment-reduce to the 128-node block via an indicator matmul
   accumulated in PSUM ([denom | numer]) -> softmax-aggregate, residual,
   conv matmul, relu -> pooling via batch-indicator matmul accumulated in PSUM.
 - Pool windows (128 graphs/core) are indirect-scattered into a global
   [3072+, 256] z buffer, AllReduced, and the readout MLP is computed
   replicated on every core.

All weights are replicated. Biases/LN affine params are applied only when
nonzero/non-one (they are zeros/ones for this model's init).
"""
import sys
import numpy as np

for _p in ("/opt/trn_rl_repo", "/root/.axon_site/_ro/trn_rl_repo"):
    if _p not in sys.path:
        sys.path.append(_p)

import concourse.bass as bass
import concourse.bacc as bacc
import concourse.mybir as mybir
import concourse.tile as tile
from concourse.bass_utils import run_bass_kernel_spmd

F32 = mybir.dt.float32
F16 = mybir.dt.float16
I32 = mybir.dt.int32
FP16_Y = True  # AllGather / gather y in fp16 (halves collective + gather bytes)
ALU = mybir.AluOpType
ACTF = mybir.ActivationFunctionType

N, E, B, D, L = 60000, 120000, 512, 256, 6
NTYPES = 25
LN_EPS = 1e-5
NC = 8
NPC = N // NC             # 7500 real nodes per core
NBLK = 61                 # 128-slot node blocks per core (bin-packed)
NSHARD = NBLK * 128       # 7808 slots per core
EDGE_CAP = 256            # target in-edges per block (2 tiles)
ZROWS = 3200              # z buffer rows (L*512 = 3072 used, row 3072 = dump)
ZDUMP = 3072

# module-level knobs (test.py pokes these; harness uses defaults)
TRACE = False
TRACE_CORES = None
LAST_RESULT = {}

_prog_cache = {}


def _ceil_div(a, b):
    return (a + b - 1) // b


# ----------------------------------------------------------------------------
# host-side preprocessing
# ----------------------------------------------------------------------------

def _prep(inputs):
    x = np.asarray(inputs["x"]).astype(np.int32).reshape(-1)
    ei = np.asarray(inputs["edge_index"]).astype(np.int64)
    ea = np.asarray(inputs["edge_attr"]).astype(np.float32).reshape(-1)
    batch = np.asarray(inputs["batch"]).astype(np.int64).reshape(-1)
    src_all, dst_all = ei[0], ei[1]

    # ---- pass 1: per-core node permutation (bin-pack by in-degree) ----
    # Nodes are re-ordered into "slots" so that each 128-slot block has a
    # near-equal number of in-edges (snake assignment by degree). Device
    # code only ever sees slot order; all index arrays are remapped here.
    glob_slot = np.zeros(N, dtype=np.int64)        # node id -> global slot row
    slot_node = []                                 # per core: slot -> node id (-1 pad)
    bin_cnts = np.zeros((NC, NBLK), dtype=np.int64)
    for c in range(NC):
        lo, hi = c * NPC, (c + 1) * NPC
        deg = np.bincount(dst_all[(dst_all >= lo) & (dst_all < hi)] - lo,
                          minlength=NPC)
        order = np.argsort(-deg, kind="stable")    # local ids, degree desc
        i = np.arange(NPC)
        chunk, pos = i // NBLK, i % NBLK
        bins = np.where(chunk % 2 == 0, pos, NBLK - 1 - pos)
        # rank within bin = chunk index (each bin gets <=123 nodes)
        cnt = np.zeros(NBLK, dtype=np.int64)
        for b in range(NBLK):
            cnt[b] = deg[order[bins == b]].sum()
        # sort bins by edge count desc so big blocks align across cores
        border = np.argsort(-cnt, kind="stable")
        bin_rank = np.empty(NBLK, dtype=np.int64)
        bin_rank[border] = np.arange(NBLK)
        new_bin = bin_rank[bins]                   # sorted-bin index per order-pos
        bin_cnts[c] = cnt[border]
        sn = np.full(NSHARD, -1, dtype=np.int64)
        sl = new_bin * 128 + chunk                 # slot per order-position
        sn[sl] = order + lo
        slot_node.append(sn)
        loc_slot = np.empty(NPC, dtype=np.int64)
        loc_slot[order] = sl
        glob_slot[lo:hi] = c * NSHARD + loc_slot

    tiles_b = np.maximum(1, _ceil_div(bin_cnts.max(axis=0), 128)).astype(np.int64)
    tile_start = np.concatenate([[0], np.cumsum(tiles_b)])
    T = int(tile_start[-1])
    T4 = _ceil_div(T, 4)
    TT = T4 * 4
    block_of_tile = np.repeat(np.arange(NBLK), tiles_b)

    # ---- pass 2: per-core edge arrays / node arrays in slot order ----
    in_maps = []
    dst_slot_all = glob_slot[dst_all]              # global slot of dst
    for c in range(NC):
        sel = (dst_slot_all >= c * NSHARD) & (dst_slot_all < (c + 1) * NSHARD)
        ds = dst_slot_all[sel] - c * NSHARD        # local slot of dst
        s = src_all[sel]
        a = ea[sel]
        blk = ds >> 7
        o = np.argsort(blk, kind="stable")
        ds, s, a, blk = ds[o], s[o], a[o], blk[o]
        cnts = np.bincount(blk, minlength=NBLK)
        assert np.all(cnts == bin_cnts[c])
        ne = len(ds)
        bstart = np.concatenate([[0], np.cumsum(cnts)])
        rank = np.arange(ne) - np.repeat(bstart[:-1], cnts)
        slot = tile_start[blk] * 128 + rank
        esrc = np.zeros(TT * 128, dtype=np.int32)
        dstl = np.full(TT * 128, -1.0, dtype=np.float32)
        eav = np.zeros(TT * 128, dtype=np.float32)
        esrc[slot] = glob_slot[s]                  # gather rows in slot space
        dstl[slot] = (ds - (blk << 7)).astype(np.float32)
        eav[slot] = a
        esrc_pm = esrc.reshape(TT, 128).T.copy()
        emeta_pm = np.concatenate(
            [dstl.reshape(TT, 128).T, eav.reshape(TT, 128).T], axis=1
        ).astype(np.float32).copy()

        # ---- node arrays in slot order ----
        sn = slot_node[c]
        valid = sn >= 0
        g0 = int(batch[c * NPC])
        bl = np.full(NSHARD, -1.0, dtype=np.float32)
        bl[valid] = (batch[sn[valid]] - g0).astype(np.float32)
        assert bl.max() < 128, "graph window exceeds 128 per core"
        batchl_pm = bl.reshape(NBLK, 128).T.copy()           # [128, NBLK]
        xid = np.zeros(NSHARD, dtype=np.int32)
        xid[valid] = x[sn[valid]]
        xid_pm = xid.reshape(NBLK, 128).T.copy()             # [128, NBLK]
        zrow_pm = np.zeros((128, L), dtype=np.int32)
        g = g0 + np.arange(128)
        for i in range(L):
            zrow_pm[:, i] = np.where(g < B, 512 * i + g, ZDUMP)

        in_maps.append(
            dict(esrc=esrc_pm, emeta=emeta_pm, batchl=batchl_pm,
                 xid=xid_pm, zrow=zrow_pm)
        )

    # ---- shared weights ----
    wl_w = np.asarray(inputs["wl_w"]).astype(np.float32)     # [L,1,D]
    conv_w = np.asarray(inputs["conv_w"]).astype(np.float32)  # [L,D,D]
    node_emb = np.asarray(inputs["node_emb"]).astype(np.float32)
    ln_scale = np.asarray(inputs["ln_scale"]).astype(np.float32)
    ln_bias = np.asarray(inputs["ln_bias"]).astype(np.float32)
    wl_b = np.asarray(inputs["wl_b"]).astype(np.float32)
    conv_b = np.asarray(inputs["conv_b"]).astype(np.float32)
    ro_w = [np.asarray(inputs[f"ro_w{i}"]).astype(np.float32) for i in range(4)]
    ro_b = [np.asarray(inputs[f"ro_b{i}"]).astype(np.float32) for i in range(4)]

    flags = dict(
        ln_affine=not (np.all(ln_scale == 1.0) and np.all(ln_bias == 0.0)),
        wl_b=bool(np.any(wl_b != 0.0)),
        conv_b=bool(np.any(conv_b != 0.0)),
        ro_b=any(np.any(b != 0.0) for b in ro_b),
    )

    shared = dict(
        wlw=np.repeat(wl_w.reshape(L, 1, D), 128, axis=1).copy(),
        convw=conv_w,
        nemb=node_emb,
        colidx=np.tile(np.arange(128, dtype=np.float32), (128, 1)).copy(),
        ident=np.eye(128, dtype=np.float32),
        row0=ro_w[0], row1=ro_w[1], row2=ro_w[2], row3=ro_w[3],
    )
    if flags["ln_affine"]:
        shared["lnsc"] = np.repeat(ln_scale.reshape(L, 1, D), 128, axis=1).copy()
        shared["lnbs"] = np.repeat(ln_bias.reshape(L, 1, D), 128, axis=1).copy()
    if flags["wl_b"]:
        shared["wlb"] = np.repeat(wl_b.reshape(L, 1, D), 128, axis=1).copy()
    if flags["conv_b"]:
        shared["convb"] = np.repeat(conv_b.reshape(L, 1, D), 128, axis=1).copy()
    if flags["ro_b"]:
        for i, b in enumerate(ro_b):
            shared[f"rob{i}"] = np.repeat(b.reshape(1, -1), 128, axis=0).copy()

    for m in in_maps:
        m.update(shared)
    return in_maps, tiles_b, block_of_tile, T, T4, flags


# ----------------------------------------------------------------------------
# device program
# ----------------------------------------------------------------------------

def _build(tiles_b, block_of_tile, T, T4, flags):
    nc = bacc.Bacc("TRN2", target_bir_lowering=False, debug=False, num_devices=NC)

    # register LN epsilon as a const AP (activation float biases need one)
    _eps_t = nc.alloc_sbuf_tensor(f"const-float32-lneps", [128, 1], F32)
    nc.gpsimd.memset(_eps_t.ap(), LN_EPS)
    nc.const_aps.aps[(F32, LN_EPS)] = _eps_t.ap()
    nc.all_engine_barrier()

    # inputs
    TT = T4 * 4
    esrc = nc.dram_tensor("esrc", [128, TT], I32, kind="ExternalInput")
    emeta = nc.dram_tensor("emeta", [128, 2 * TT], F32, kind="ExternalInput")
    batchl = nc.dram_tensor("batchl", [128, NBLK], F32, kind="ExternalInput")
    xid = nc.dram_tensor("xid", [128, NBLK], I32, kind="ExternalInput")
    zrow = nc.dram_tensor("zrow", [128, L], I32, kind="ExternalInput")
    wlw = nc.dram_tensor("wlw", [L, 128, D], F32, kind="ExternalInput")
    convw = nc.dram_tensor("convw", [L, D, D], F32, kind="ExternalInput")
    nemb = nc.dram_tensor("nemb", [NTYPES, D], F32, kind="ExternalInput")
    colidx = nc.dram_tensor("colidx", [128, 128], F32, kind="ExternalInput")
    ident = nc.dram_tensor("ident", [128, 128], F32, kind="ExternalInput")
    row0 = nc.dram_tensor("row0", [6 * D, 768], F32, kind="ExternalInput")
    row1 = nc.dram_tensor("row1", [768, 384], F32, kind="ExternalInput")
    row2 = nc.dram_tensor("row2", [384, 192], F32, kind="ExternalInput")
    row3 = nc.dram_tensor("row3", [192, 1], F32, kind="ExternalInput")
    lnsc = lnbs = wlb = convb = None
    if flags["ln_affine"]:
        lnsc = nc.dram_tensor("lnsc", [L, 128, D], F32, kind="ExternalInput")
        lnbs = nc.dram_tensor("lnbs", [L, 128, D], F32, kind="ExternalInput")
    if flags["wl_b"]:
        wlb = nc.dram_tensor("wlb", [L, 128, D], F32, kind="ExternalInput")
    if flags["conv_b"]:
        convb = nc.dram_tensor("convb", [L, 128, D], F32, kind="ExternalInput")
    robs = None
    if flags["ro_b"]:
        robs = [
            nc.dram_tensor(f"rob{i}", [128, n], F32, kind="ExternalInput")
            for i, n in enumerate([768, 384, 192, 1])
        ]

    out = nc.dram_tensor("out", [B, 1], F32, kind="ExternalOutput")

    # LN groups of up to 4 blocks
    groups = []
    b0 = 0
    while b0 < NBLK:
        nb = min(4, NBLK - b0)
        groups.append((b0, nb))
        b0 += nb

    # tile ranges per block
    tstart = np.concatenate([[0], np.cumsum(tiles_b)]).astype(int)

    with tile.TileContext(nc) as tc:
        with (
            tc.tile_pool(name="dram", bufs=1, space="DRAM") as dram,
            tc.tile_pool(name="consts", bufs=1) as cpool,
            tc.tile_pool(name="lweights", bufs=2) as wpool,
        ):
            YDT = F16 if FP16_Y else F32
            hA = dram.tile([NSHARD, D], F32, tag="hA")
            hB = dram.tile([NSHARD, D], F32, tag="hB")
            y_c = dram.tile([NSHARD, D], YDT, tag="y_c")
            y_fulls = [
                dram.tile([NC * NSHARD, D], YDT, tag=f"y_full_{i}",
                          name=f"y_full_{i}", addr_space="Shared")
                for i in range(L)
            ]
            z_all = dram.tile([ZROWS, D], F32, tag="z_all")
            z_red = dram.tile([ZROWS, D], F32, tag="z_red", addr_space="Shared")

            colidx_t = cpool.tile([128, 128], F32, tag="colidx")
            nc.sync.dma_start(out=colidx_t[:], in_=colidx[:])
            ident_t = cpool.tile([128, 128], F32, tag="ident")
            nc.sync.dma_start(out=ident_t[:], in_=ident[:])
            # whole-kernel metadata, loaded once (partition-major)
            esrc_sb = cpool.tile([128, TT], I32, tag="esrc_sb")
            nc.sync.dma_start(out=esrc_sb[:], in_=esrc[:])
            emeta_sb = cpool.tile([128, 2 * TT], F32, tag="emeta_sb")
            nc.sync.dma_start(out=emeta_sb[:], in_=emeta[:])
            batchl_sb = cpool.tile([128, NBLK], F32, tag="batchl_sb")
            nc.sync.dma_start(out=batchl_sb[:], in_=batchl[:])
            xid_sb = cpool.tile([128, NBLK], I32, tag="xid_sb")
            nc.sync.dma_start(out=xid_sb[:], in_=xid[:])
            zrow_sb = cpool.tile([128, L], I32, tag="zrow_sb")
            nc.sync.dma_start(out=zrow_sb[:], in_=zrow[:])
            # pooling indicators, built once (reused across layers)
            ind_ts = []
            for b in range(NBLK):
                ind_b = cpool.tile([128, 128], F32, tag=f"ind{b}",
                                   name=f"ind{b}")
                nc.vector.tensor_tensor(
                    out=ind_b[:],
                    in0=batchl_sb[:, b:b + 1].to_broadcast([128, 128]),
                    in1=colidx_t[:],
                    op=ALU.is_equal,
                )
                ind_ts.append(ind_b)
            # segment-indicator matrices, built once (layer-invariant)
            st_ts = []
            for t in range(T):
                st_t = cpool.tile([128, 128], F32, tag=f"st{t}", name=f"st{t}")
                nc.vector.tensor_tensor(
                    out=st_t[:],
                    in0=emeta_sb[:, t:t + 1].to_broadcast([128, 128]),
                    in1=colidx_t[:],
                    op=ALU.is_equal,
                )
                st_ts.append(st_t)

            # ---------- zero z_all ----------
            with tc.tile_pool(name="zz", bufs=1) as zz:
                zt = zz.tile([128, D], F32)
                nc.vector.memset(zt[:], 0.0)
                for k in range(ZROWS // 128):
                    nc.sync.dma_start(
                        out=z_all[k * 128:(k + 1) * 128, :], in_=zt[:]
                    )

            # ---------- embedding ----------
            with tc.tile_pool(name="emb", bufs=4) as ep:
                for b in range(NBLK):
                    et = ep.tile([128, D], F32, tag="et")
                    nc.gpsimd.indirect_dma_start(
                        out=et[:], out_offset=None, in_=nemb[:],
                        in_offset=bass.IndirectOffsetOnAxis(
                            ap=xid_sb[:, b:b + 1], axis=0
                        ),
                    )
                    nc.sync.dma_start(
                        out=hA[b * 128:(b + 1) * 128, :], in_=et[:]
                    )

            # ---------- layers ----------
            with (
                tc.tile_pool(name="ln", bufs=2) as lp,
                tc.tile_pool(name="edge", bufs=6) as xp,
                tc.tile_pool(name="blk", bufs=3) as bp,
                tc.tile_pool(name="ps_nd", bufs=3, space="PSUM") as ps_nd,
                tc.tile_pool(name="ps_xt", bufs=2, space="PSUM") as ps_xt,
                tc.tile_pool(name="ps_h", bufs=1, space="PSUM") as ps_h,
                tc.tile_pool(name="ps_pool", bufs=1, space="PSUM") as ps_pool,
            ):
                for li in range(L):
                    h_in = hA if li % 2 == 0 else hB
                    h_out = hB if li % 2 == 0 else hA

                    wlw_t = wpool.tile([128, D], F32, tag="wlw")
                    nc.sync.dma_start(out=wlw_t[:], in_=wlw[li])
                    cw0 = wpool.tile([128, D], F32, tag="cw0")
                    nc.sync.dma_start(out=cw0[:], in_=convw[li, 0:128, :])
                    cw1 = wpool.tile([128, D], F32, tag="cw1")
                    nc.sync.dma_start(out=cw1[:], in_=convw[li, 128:256, :])
                    if flags["ln_affine"]:
                        lnsc_t = wpool.tile([128, D], F32, tag="lnsc")
                        nc.sync.dma_start(out=lnsc_t[:], in_=lnsc[li])
                        lnbs_t = wpool.tile([128, D], F32, tag="lnbs")
                        nc.sync.dma_start(out=lnbs_t[:], in_=lnbs[li])
                    if flags["wl_b"]:
                        wlb_t = wpool.tile([128, D], F32, tag="wlb")
                        nc.sync.dma_start(out=wlb_t[:], in_=wlb[li])
                    if flags["conv_b"]:
                        convb_t = wpool.tile([128, D], F32, tag="convb")
                        nc.sync.dma_start(out=convb_t[:], in_=convb[li])

                    # ---- LayerNorm: h_in -> y_c ----
                    for (gb, nb) in groups:
                        rows = slice(gb * 128, (gb + nb) * 128)
                        h_ap = h_in[rows, :].rearrange("(j p) d -> p j d", p=128)
                        ht = lp.tile([128, nb, D], F32, tag="ht")
                        nc.sync.dma_start(out=ht[:], in_=h_ap)
                        # mean and E[x^2] via two reductions; var = E[x^2]-mu^2
                        mu = lp.tile([128, 4], F32, tag="mu")
                        nc.vector.tensor_reduce(
                            out=mu[:, :nb], in_=ht[:], axis=mybir.AxisListType.X,
                            op=ALU.add,
                        )
                        mus = lp.tile([128, 4], F32, tag="mus")
                        nc.scalar.mul(mus[:, :nb], mu[:, :nb], 1.0 / D)
                        sq = lp.tile([128, nb, D], F32, tag="sq")
                        nc.scalar.square(sq[:], ht[:])
                        vs = lp.tile([128, 4], F32, tag="vs")
                        nc.vector.tensor_reduce(
                            out=vs[:, :nb], in_=sq[:], axis=mybir.AxisListType.X,
                            op=ALU.add,
                        )
                        msq = lp.tile([128, 4], F32, tag="msq")
                        nc.vector.tensor_tensor(
                            out=msq[:, :nb], in0=mus[:, :nb], in1=mus[:, :nb],
                            op=ALU.mult,
                        )
                        vv = lp.tile([128, 4], F32, tag="vv")
                        nc.vector.tensor_scalar(
                            out=vv[:, :nb], in0=vs[:, :nb],
                            scalar1=1.0 / D, scalar2=None, op0=ALU.mult,
                        )
                        nc.vector.tensor_tensor(
                            out=vv[:, :nb], in0=vv[:, :nb], in1=msq[:, :nb],
                            op=ALU.subtract,
                        )
                        sd = lp.tile([128, 4], F32, tag="sd")
                        nc.scalar.activation(
                            sd[:, :nb], vv[:, :nb], ACTF.Sqrt, bias=LN_EPS
                        )
                        rs = lp.tile([128, 4], F32, tag="rs")
                        nc.vector.reciprocal(rs[:, :nb], sd[:, :nb])
                        yt = lp.tile([128, nb, D], YDT, tag="yt")
                        for j in range(nb):
                            nc.vector.tensor_scalar(
                                out=yt[:, j, :], in0=ht[:, j, :],
                                scalar1=mus[:, j:j + 1], scalar2=rs[:, j:j + 1],
                                op0=ALU.subtract, op1=ALU.mult,
                            )
                        if flags["ln_affine"]:
                            nc.vector.tensor_tensor(
                                out=yt[:], in0=yt[:],
                                in1=lnsc_t[:, None, :].to_broadcast([128, nb, D]),
                                op=ALU.mult,
                            )
                            nc.vector.tensor_tensor(
                                out=yt[:], in0=yt[:],
                                in1=lnbs_t[:, None, :].to_broadcast([128, nb, D]),
                                op=ALU.add,
                            )
                        y_ap = y_c[rows, :].rearrange("(j p) d -> p j d", p=128)
                        nc.sync.dma_start(out=y_ap, in_=yt[:])

                    # ---- AllGather y ----
                    y_full = y_fulls[li]
                    nc.gpsimd.collective_compute(
                        "AllGather", ALU.bypass,
                        replica_groups=[list(range(NC))],
                        ins=[y_c[:].opt()], outs=[y_full[:].opt()],
                    )

                    # ---- edges + conv + pool ----
                    ppool = ps_pool.tile([128, D], F32, tag="ppool")
                    for b in range(NBLK):
                        nd = ps_nd.tile([128, 2 * D], F32, tag="nd")
                        t0, t1 = int(tstart[b]), int(tstart[b + 1])
                        for t in range(t0, t1):
                            ysrc = xp.tile([128, D], YDT, tag="ysrc")
                            nc.gpsimd.indirect_dma_start(
                                out=ysrc[:], out_offset=None, in_=y_full[:],
                                in_offset=bass.IndirectOffsetOnAxis(
                                    ap=esrc_sb[:, t:t + 1], axis=0
                                ),
                            )
                            pre = xp.tile([128, D], F32, tag="pre")
                            nc.vector.scalar_tensor_tensor(
                                out=pre[:], in0=wlw_t[:],
                                scalar=emeta_sb[:, TT + t:TT + t + 1], in1=ysrc[:],
                                op0=ALU.mult, op1=ALU.add,
                            )
                            if flags["wl_b"]:
                                nc.vector.tensor_tensor(
                                    out=pre[:], in0=pre[:], in1=wlb_t[:],
                                    op=ALU.add,
                                )
                            msg = xp.tile([128, D], F32, tag="msg")
                            nc.scalar.activation(msg[:], pre[:], ACTF.Relu)
                            ev = xp.tile([128, 2 * D], F32, tag="ev")
                            nc.scalar.activation(ev[:, :D], msg[:], ACTF.Exp)
                            nc.vector.tensor_tensor(
                                out=ev[:, D:], in0=msg[:], in1=ev[:, :D],
                                op=ALU.mult,
                            )
                            nc.tensor.matmul(
                                out=nd[:], lhsT=st_ts[t][:], rhs=ev[:],
                                start=(t == t0), stop=(t == t1 - 1),
                            )
                        # block post: softmax-agg + residual + conv + pool
                        td = bp.tile([128, D], F32, tag="td")
                        nc.vector.tensor_scalar_max(td[:], nd[:, :D], 1e-16)
                        yb = bp.tile([128, D], YDT, tag="yb")
                        nc.sync.dma_start(
                            out=yb[:], in_=y_c[b * 128:(b + 1) * 128, :]
                        )
                        rec = bp.tile([128, D], F32, tag="rec")
                        nc.vector.reciprocal_approx_fast(out=rec[:], in_=td[:])
                        xv = bp.tile([128, D], F32, tag="xv")
                        nc.vector.tensor_tensor(
                            out=xv[:], in0=nd[:, D:], in1=rec[:], op=ALU.mult
                        )
                        nc.vector.tensor_tensor(
                            out=xv[:], in0=xv[:], in1=yb[:], op=ALU.add
                        )
                        pxt = ps_xt.tile([128, D], F32, tag="pxt")
                        nc.tensor.transpose(
                            out=pxt[:, 0:128], in_=xv[:, 0:128], identity=ident_t[:]
                        )
                        nc.tensor.transpose(
                            out=pxt[:, 128:256], in_=xv[:, 128:256],
                            identity=ident_t[:],
                        )
                        xts = bp.tile([128, D], F32, tag="xts")
                        nc.vector.tensor_copy(out=xts[:], in_=pxt[:])
                        ph = ps_h.tile([128, D], F32, tag="ph")
                        nc.tensor.matmul(
                            out=ph[:], lhsT=xts[:, 0:128], rhs=cw0[:],
                            start=True, stop=False,
                        )
                        nc.tensor.matmul(
                            out=ph[:], lhsT=xts[:, 128:256], rhs=cw1[:],
                            start=False, stop=True,
                        )
                        if flags["conv_b"]:
                            nc.vector.tensor_tensor(
                                out=ph[:], in0=ph[:], in1=convb_t[:], op=ALU.add
                            )
                        hb = bp.tile([128, D], F32, tag="hb")
                        nc.scalar.activation(hb[:], ph[:], ACTF.Relu)
                        nc.sync.dma_start(
                            out=h_out[b * 128:(b + 1) * 128, :], in_=hb[:]
                        )
                        nc.tensor.matmul(
                            out=ppool[:], lhsT=ind_ts[b][:], rhs=hb[:],
                            start=(b == 0), stop=(b == NBLK - 1),
                        )
                    # pool -> z_all
                    zp = bp.tile([128, D], F32, tag="zp")
                    nc.vector.tensor_copy(out=zp[:], in_=ppool[:])
                    nc.gpsimd.indirect_dma_start(
                        out=z_all[:],
                        out_offset=bass.IndirectOffsetOnAxis(
                            ap=zrow_sb[:, li:li + 1], axis=0
                        ),
                        in_=zp[:], in_offset=None,
                    )

            # ---------- AllReduce z ----------
            nc.gpsimd.collective_compute(
                "AllReduce", ALU.add,
                replica_groups=[list(range(NC))],
                ins=[z_all[:].opt()], outs=[z_red[:].opt()],
            )

            # ---------- readout MLP (replicated) ----------
            with (
                tc.tile_pool(name="row", bufs=1) as rw,
                tc.tile_pool(name="ro", bufs=2) as ro,
                tc.tile_pool(name="ps_a", bufs=1, space="PSUM") as psa,
                tc.tile_pool(name="ps_b", bufs=1, space="PSUM") as psb,
                tc.tile_pool(name="ps_t", bufs=2, space="PSUM") as pst,
                tc.tile_pool(name="ps_o", bufs=1, space="PSUM") as pso,
            ):
                w0t = []
                for f in range(12):
                    w = rw.tile([128, 768], F32, tag=f"w0_{f}")
                    nc.sync.dma_start(out=w[:], in_=row0[f * 128:(f + 1) * 128, :])
                    w0t.append(w)
                w1t = []
                for f in range(6):
                    w = rw.tile([128, 384], F32, tag=f"w1_{f}")
                    nc.sync.dma_start(out=w[:], in_=row1[f * 128:(f + 1) * 128, :])
                    w1t.append(w)
                w2t = []
                for f in range(3):
                    w = rw.tile([128, 192], F32, tag=f"w2_{f}")
                    nc.sync.dma_start(out=w[:], in_=row2[f * 128:(f + 1) * 128, :])
                    w2t.append(w)
                w3a = rw.tile([128, 1], F32, tag="w3a")
                nc.sync.dma_start(out=w3a[:], in_=row3[0:128, :])
                w3b = rw.tile([64, 1], F32, tag="w3b")
                nc.sync.dma_start(out=w3b[:], in_=row3[128:192, :])
                robt = []
                if flags["ro_b"]:
                    for i, n in enumerate([768, 384, 192, 1]):
                        w = rw.tile([128, n], F32, tag=f"rob{i}")
                        nc.sync.dma_start(out=w[:], in_=robs[i][:])
                        robt.append(w)

                def transpose_chunk(src_ap, kdim):
                    """src_ap: [128, kdim] SBUF -> returns [kdim,128] SBUF tile."""
                    pt = pst.tile([128, 128], F32, tag="pt")
                    nc.tensor.transpose(
                        out=pt[:kdim, :], in_=src_ap, identity=ident_t[:]
                    )
                    ct = ro.tile([128, 128], F32, tag="ct")
                    nc.vector.tensor_copy(out=ct[:kdim, :], in_=pt[:kdim, :])
                    return ct

                for gb in range(4):
                    pA = psa.tile([128, 512], F32, tag="pA")
                    pB = psb.tile([128, 256], F32, tag="pB")
                    for f in range(12):
                        li, half = f // 2, f % 2
                        zc = ro.tile([128, 128], F32, tag="zc")
                        nc.sync.dma_start(
                            out=zc[:],
                            in_=z_red[
                                512 * li + 128 * gb: 512 * li + 128 * (gb + 1),
                                128 * half: 128 * (half + 1),
                            ],
                        )
                        zt = transpose_chunk(zc[:], 128)
                        nc.tensor.matmul(
                            out=pA[:], lhsT=zt[:], rhs=w0t[f][:, 0:512],
                            start=(f == 0), stop=(f == 11),
                        )
                        nc.tensor.matmul(
                            out=pB[:], lhsT=zt[:], rhs=w0t[f][:, 512:768],
                            start=(f == 0), stop=(f == 11),
                        )
                    z1 = ro.tile([128, 768], F32, tag="z1")
                    if flags["ro_b"]:
                        nc.vector.tensor_tensor(
                            out=pA[:], in0=pA[:], in1=robt[0][:, 0:512], op=ALU.add
                        )
                        nc.vector.tensor_tensor(
                            out=pB[:], in0=pB[:], in1=robt[0][:, 512:768], op=ALU.add
                        )
                    nc.scalar.activation(z1[:, 0:512], pA[:], ACTF.Gelu)
                    nc.scalar.activation(z1[:, 512:768], pB[:], ACTF.Gelu)

                    p2 = psa.tile([128, 384], F32, tag="p2")
                    for f in range(6):
                        zt = transpose_chunk(z1[:, 128 * f:128 * (f + 1)], 128)
                        nc.tensor.matmul(
                            out=p2[:], lhsT=zt[:], rhs=w1t[f][:],
                            start=(f == 0), stop=(f == 5),
                        )
                    if flags["ro_b"]:
                        nc.vector.tensor_tensor(
                            out=p2[:], in0=p2[:], in1=robt[1][:], op=ALU.add
                        )
                    z2 = ro.tile([128, 384], F32, tag="z2")
                    nc.scalar.activation(z2[:], p2[:], ACTF.Gelu)

                    p3 = psb.tile([128, 192], F32, tag="p3")
                    for f in range(3):
                        zt = transpose_chunk(z2[:, 128 * f:128 * (f + 1)], 128)
                        nc.tensor.matmul(
                            out=p3[:], lhsT=zt[:], rhs=w2t[f][:],
                            start=(f == 0), stop=(f == 2),
                        )
                    if flags["ro_b"]:
                        nc.vector.tensor_tensor(
                            out=p3[:], in0=p3[:], in1=robt[2][:], op=ALU.add
                        )
                    z3 = ro.tile([128, 192], F32, tag="z3")
                    nc.scalar.activation(z3[:], p3[:], ACTF.Gelu)

                    po = pso.tile([128, 1], F32, tag="po")
                    zt = transpose_chunk(z3[:, 0:128], 128)
                    nc.tensor.matmul(
                        out=po[:], lhsT=zt[:], rhs=w3a[:],
                        start=True, stop=False,
                    )
                    zt = transpose_chunk(z3[:, 128:192], 64)
                    nc.tensor.matmul(
                        out=po[:], lhsT=zt[:64, :], rhs=w3b[:],
                        start=False, stop=True,
                    )
                    oc = ro.tile([128, 1], F32, tag="oc")
                    if flags["ro_b"]:
                        nc.vector.tensor_tensor(
                            out=po[:], in0=po[:], in1=robt[3][:], op=ALU.add
                        )
                    nc.vector.tensor_copy(out=oc[:], in_=po[:])
                    nc.sync.dma_start(
                        out=out[128 * gb:128 * (gb + 1), :], in_=oc[:]
                    )

    nc.compile()
    return nc


# ----------------------------------------------------------------------------
# entry point
# ----------------------------------------------------------------------------

def kernel(**inputs):
    in_maps, tiles_b, block_of_tile, T, T4, flags = _prep(inputs)
    key = (tuple(tiles_b.tolist()), tuple(sorted(flags.items())))
    if key not in _prog_cache:
        _prog_cache[key] = _build(tiles_b, block_of_tile, T, T4, flags)
    nc = _prog_cache[key]

    kwargs = {}
    if TRACE:
        kwargs = dict(trace=True, trace_cores=TRACE_CORES)
    res = run_bass_kernel_spmd(nc, in_maps, list(range(NC)), **kwargs)
    LAST_RESULT["exec_time_ns"] = getattr(res, "exec_time_ns", None)
    LAST_RESULT["res"] = res
    return np.asarray(res.results[0]["out"], dtype=np.float32)



# revision 15
# speedup vs baseline: 1.1338x; 1.1338x over previous
"""Trainium2 Bass kernel for GCNNetwork (GENConv message passing, L=6).

Strategy (graph-data parallel over 8 NeuronCores):
 - Nodes sharded contiguously; per core, nodes are permuted into 61 blocks of
   128 slots. Blocks 0..30 ("chunk A") hold the highest out-degree nodes so
   most edge sources live in chunk A; blocks 31..60 are chunk B.
 - Per layer, LayerNorm is fused into the previous layer's block loop: as soon
   as a block's h is produced, its stats (via activation accum) and normalized
   y are computed and written to y_c. The y AllGather is split in two: chunk A
   fires mid-loop (after block 30) and overlaps the rest of the block loop;
   chunk B fires at the loop end and overlaps the next layer's A-only tiles
   (tile 0 of every block gathers only chunk-A rows).
 - Edge aggregation: per 128-edge tile, indirect-gather y[src] (fp16), compute
   msg/exp/msg*exp, segment-reduce to the block's 128 nodes via an indicator
   matmul accumulated in PSUM ([denom | numer]); then softmax-agg, residual,
   conv matmul, relu, and pooling via a batch-indicator matmul.
 - Pooling stays device-local: batch is sorted, so each core's nodes span a
   ~64-graph window. Only the partial sums of the single boundary graph are
   exchanged (one tiny AllGather); each core runs the readout MLP for the
   ~64 graphs it owns, and a [128,1] AllGather + index map assembles the
   final [512,1] output on every core.
 - All matmul operands are fp16 (PSUM accumulation in fp32).
"""
import sys
import numpy as np

for _p in ("/opt/trn_rl_repo", "/root/.axon_site/_ro/trn_rl_repo"):
    if _p not in sys.path:
        sys.path.append(_p)

import ml_dtypes
import concourse.bass as bass
import concourse.bacc as bacc
import concourse.mybir as mybir
import concourse.tile as tile
from concourse.bass_utils import run_bass_kernel_spmd

F32 = mybir.dt.float32
F16 = mybir.dt.float16
I32 = mybir.dt.int32
ALU = mybir.AluOpType
ACTF = mybir.ActivationFunctionType
BF16NP = ml_dtypes.bfloat16  # unused for now; fp16 everywhere

N, E, B, D, L = 60000, 120000, 512, 256, 6
NTYPES = 25
LN_EPS = 1e-5
NC = 8
NPC = N // NC             # 7500 real nodes per core
NBLK = 61                 # 128-slot node blocks per core
NA = 31                   # chunk-A blocks (high out-degree nodes)
NB_ = NBLK - NA           # chunk-B blocks
NSLOT_A = NA * 128        # 3968 (== A real nodes per core, blocks full)
NSLOT_B = NB_ * 128       # 3840 (3532 real + pad)
NSHARD = NBLK * 128       # 7808 slots per core
GA_ROWS = NC * NSLOT_A    # y_full rows holding chunk A of all cores
GTOT = NC * NSHARD
ZG = 96                   # max owned graphs per core (padded)
ZROWS_L = ZG * L + 1      # local z rows (+1 dump)
ZDUMP = ZG * L

# module-level knobs (test.py pokes these; harness uses defaults)
TRACE = False
TRACE_CORES = None
LAST_RESULT = {}

_prog_cache = {}


def _ceil_div(a, b):
    return (a + b - 1) // b


# ----------------------------------------------------------------------------
# host-side preprocessing
# ----------------------------------------------------------------------------

def _prep(inputs):
    x = np.asarray(inputs["x"]).astype(np.int32).reshape(-1)
    ei = np.asarray(inputs["edge_index"]).astype(np.int64)
    ea = np.asarray(inputs["edge_attr"]).astype(np.float32).reshape(-1)
    batch = np.asarray(inputs["batch"]).astype(np.int64).reshape(-1)
    src_all, dst_all = ei[0], ei[1]
    outdeg = np.bincount(src_all, minlength=N)

    # ---- pass 1: per-core node permutation ----
    # Top NSLOT_A nodes by out-degree go to blocks 0..NA-1 (chunk A), rest to
    # chunk B; a node's chunk decides which AllGather carries its y row. Every
    # edge tile is A-only or B-only by SOURCE chunk, so B tiles are the only
    # ones waiting on the second AllGather. Nodes are packed into blocks by a
    # greedy 2D heuristic equalizing both A-src and B-src in-edge counts.
    slot_node = []            # per core: slot -> node id (-1 pad)
    loc_slot_all = np.full(N, -1, dtype=np.int64)
    yrow = np.full(N, -1, dtype=np.int64)             # node -> y_full[AB] row

    src_of = src_all
    # in-degree split by src chunk requires src chunk first: compute A-set
    # membership globally (per owning core) before packing dst blocks.
    a_node = np.zeros(N, dtype=bool)
    for c in range(NC):
        lo = c * NPC
        od = outdeg[lo:lo + NPC]
        order = np.argsort(-od, kind="stable")
        a_node[lo + order[:NSLOT_A]] = True
    src_is_a_e = a_node[src_all]

    # per-node in-degree split by src chunk
    da_all = np.zeros(N, dtype=np.int64)
    db_all = np.zeros(N, dtype=np.int64)
    np.add.at(da_all, dst_all[src_is_a_e], 1)
    np.add.at(db_all, dst_all[~src_is_a_e], 1)

    # B-free prefix: blocks whose nodes have zero B-src in-edges. At the next
    # layer's start these blocks only need AG_A, so AG_B hides behind them.
    nq = [int(np.sum((a_node & (db_all == 0))[c * NPC:(c + 1) * NPC]))
          for c in range(NC)]
    NPFX = min(min(q // 128 for q in nq), NA - 4)

    def greedy(da, db, ids, nblk, cap):
        """Assign ids to nblk bins (<=cap each), equalizing (eA, eB)."""
        o = np.argsort(-(da[ids] + db[ids]), kind="stable")
        ids = ids[o]
        gea = np.zeros(nblk)
        geb = np.zeros(nblk)
        used = np.zeros(nblk, dtype=np.int64)
        bin_of = np.empty(len(ids), dtype=np.int64)
        for i in range(len(ids)):
            nid = ids[i]
            cost = np.maximum(gea + da[nid], geb + db[nid])
            cost[used >= cap] = np.inf
            bb = int(np.argmin(cost))
            bin_of[i] = bb
            gea[bb] += da[nid]
            geb[bb] += db[nid]
            used[bb] += 1
        return ids, bin_of, gea, geb

    blk_ea = np.zeros((NC, NBLK), dtype=np.int64)
    blk_eb = np.zeros((NC, NBLK), dtype=np.int64)
    for c in range(NC):
        lo = c * NPC
        da = da_all[lo:lo + NPC]
        db = db_all[lo:lo + NPC]
        od = outdeg[lo:lo + NPC]
        order = np.argsort(-od, kind="stable")
        a_ids = order[:NSLOT_A]
        b_ids = order[NSLOT_A:]
        qmask = db[a_ids] == 0
        q_ids = a_ids[qmask]
        q_ids = q_ids[np.argsort(-da[q_ids], kind="stable")]
        pfx_ids = q_ids[:NPFX * 128]
        rest_a = np.concatenate([q_ids[NPFX * 128:], a_ids[~qmask]])
        sn = np.full(NSHARD, -1, dtype=np.int64)
        for ids, nblk, blk0, cap in (
            (pfx_ids, NPFX, 0, 128),
            (rest_a, NA - NPFX, NPFX, 128),
            (b_ids, NB_, NA, _ceil_div(len(b_ids), NB_)),
        ):
            ids, bin_of, gea, geb = greedy(da, db, ids, nblk, cap)
            # order bins by eA desc to align heavy blocks across cores
            border = np.argsort(-(gea * 1e6 + geb), kind="stable")
            bin_rank = np.empty(nblk, dtype=np.int64)
            bin_rank[border] = np.arange(nblk)
            nb2 = bin_rank[bin_of]
            # slot within block = arrival order per bin
            slot_in = np.zeros(len(ids), dtype=np.int64)
            ctr = np.zeros(nblk, dtype=np.int64)
            for i in range(len(ids)):
                slot_in[i] = ctr[nb2[i]]
                ctr[nb2[i]] += 1
            slot = (nb2 + blk0) * 128 + slot_in
            sn[slot] = ids + lo
            loc_slot_all[ids + lo] = slot
            for b in range(nblk):
                blk_ea[c, b + blk0] = gea[border[b]]
                blk_eb[c, b + blk0] = geb[border[b]]
        slot_node.append(sn)
        s = loc_slot_all[lo:lo + NPC]
        yrow[lo:lo + NPC] = np.where(
            s < NSLOT_A,
            c * NSLOT_A + s,
            GA_ROWS + c * NSLOT_B + (s - NSLOT_A),
        )

    dst_slot = loc_slot_all[dst_all]
    dst_core = dst_all // NPC

    # tiles per block: A tiles first, then B tiles (counts are cross-core max)
    na_tiles = np.maximum(1, _ceil_div(blk_ea.max(axis=0), 128))
    nb_tiles = _ceil_div(blk_eb.max(axis=0), 128)
    assert np.all(nb_tiles[:NPFX] == 0)
    tiles_b = na_tiles + nb_tiles
    tstart = np.concatenate([[0], np.cumsum(tiles_b)]).astype(int)
    T = int(tstart[-1])
    T4 = _ceil_div(T, 4)
    TT = T4 * 4

    # ---- graph ownership / readout maps ----
    fg = np.array([batch[c * NPC] for c in range(NC)] + [B], dtype=np.int64)
    own0 = np.empty(NC + 1, dtype=np.int64)
    own0[0] = 0
    own0[NC] = B
    for c in range(1, NC):
        # graph fg[c] is owned by core c-1 if it started there
        own0[c] = fg[c] + 1 if batch[c * NPC - 1] == fg[c] else fg[c]
    wown = own0[1:] - own0[:-1]
    assert wown.max() <= ZG, f"owned graphs {wown.max()} > {ZG}"

    amap = np.zeros((128, 4), dtype=np.int32)
    for g in range(B):
        o = int(np.searchsorted(own0[1:], g, side="right"))
        amap[g % 128, g // 128] = o * 128 + (g - own0[o])

    # ---- pass 2: per-core arrays ----
    in_maps = []
    for c in range(NC):
        lo = c * NPC
        sel = dst_core == c
        ds = dst_slot[sel]
        s = src_all[sel]
        a = ea[sel]
        isa = src_is_a_e[sel]
        blk = ds >> 7
        o = np.lexsort((~isa, blk))                   # by block, A-src first
        ds, s, a, blk, isa = ds[o], s[o], a[o], blk[o], isa[o]
        bstart = np.concatenate([[0], np.cumsum(np.bincount(blk, minlength=NBLK))])

        esrc = np.zeros(TT * 128, dtype=np.int32)
        dstl = np.full(TT * 128, -1.0, dtype=np.float32)
        eav = np.zeros(TT * 128, dtype=np.float32)
        for b in range(NBLK):
            e0, e1 = int(bstart[b]), int(bstart[b + 1])
            na_e = int(blk_ea[c, b])
            assert na_e <= na_tiles[b] * 128
            assert (e1 - e0 - na_e) <= nb_tiles[b] * 128
            baseA = tstart[b] * 128
            baseB = (tstart[b] + na_tiles[b]) * 128
            idx = np.concatenate([
                np.arange(baseA, baseA + na_e),
                np.arange(baseB, baseB + (e1 - e0 - na_e)),
            ])
            ee = np.arange(e0, e1)
            rows = yrow[s[ee]]
            rows = np.where(rows < GA_ROWS, rows, rows - GA_ROWS)
            esrc[idx] = rows
            dstl[idx] = (ds[ee] - (blk[ee] << 7)).astype(np.float32)
            eav[idx] = a[ee]
        esrc_pm = esrc.reshape(TT, 128).T.copy()
        emeta_pm = np.concatenate(
            [dstl.reshape(TT, 128).T, eav.reshape(TT, 128).T], axis=1
        ).astype(np.float32).copy()

        # node arrays in slot order
        sn = slot_node[c]
        valid = sn >= 0
        g0 = int(batch[lo])
        bl = np.full(NSHARD, -1.0, dtype=np.float32)
        bl[valid] = (batch[sn[valid]] - g0).astype(np.float32)
        assert bl.max() < 128, "graph window exceeds 128 per core"
        batchl_pm = bl.reshape(NBLK, 128).T.copy()
        xid = np.zeros(NSHARD, dtype=np.int32)
        xid[valid] = x[sn[valid]]
        xid_pm = xid.reshape(NBLK, 128).T.copy()

        # zrow: pool-window row p (graph g0+p) -> local z row (q*L+li) or dump
        zrow_pm = np.full((128, L), ZDUMP, dtype=np.int32)
        for p in range(128):
            g = g0 + p
            if own0[c] <= g < own0[c + 1]:
                q = g - own0[c]
                for li in range(L):
                    zrow_pm[p, li] = q * L + li

        # boundary add: if graph fg[c+1] is owned by this core, add core c+1's
        # xchg partial rows into local z rows of that graph
        ztgt = np.full((8, 1), ZDUMP, dtype=np.int32)
        xrow = np.zeros((8, 1), dtype=np.int32)
        if c < NC - 1 and own0[c + 1] == fg[c + 1] + 1:
            q = int(fg[c + 1] - own0[c])
            for li in range(L):
                ztgt[li, 0] = q * L + li
                xrow[li, 0] = (c + 1) * L + li

        in_maps.append(dict(
            esrc=esrc_pm, emeta=emeta_pm, batchl=batchl_pm, xid=xid_pm,
            zrow=zrow_pm, ztgt=ztgt, xrow=xrow,
        ))

    # ---- shared weights ----
    f16 = np.float16
    wl_w = np.asarray(inputs["wl_w"]).astype(np.float32)      # [L,1,D]
    conv_w = np.asarray(inputs["conv_w"]).astype(np.float32)  # [L,D,D]
    node_emb = np.asarray(inputs["node_emb"]).astype(np.float32)
    ln_scale = np.asarray(inputs["ln_scale"]).astype(np.float32)
    ln_bias = np.asarray(inputs["ln_bias"]).astype(np.float32)
    wl_b = np.asarray(inputs["wl_b"]).astype(np.float32)
    conv_b = np.asarray(inputs["conv_b"]).astype(np.float32)
    ro_w = [np.asarray(inputs[f"ro_w{i}"]).astype(np.float32) for i in range(4)]
    ro_b = [np.asarray(inputs[f"ro_b{i}"]).astype(np.float32) for i in range(4)]

    flags = dict(
        ln_affine=not (np.all(ln_scale == 1.0) and np.all(ln_bias == 0.0)),
        wl_b=bool(np.any(wl_b != 0.0)),
        conv_b=bool(np.any(conv_b != 0.0)),
        ro_b=any(np.any(b != 0.0) for b in ro_b),
    )

    shared = dict(
        wlw=np.repeat(wl_w.reshape(L, 1, D), 128, axis=1).astype(f16).copy(),
        convw=conv_w.astype(f16),
        nemb=node_emb,
        colidx=np.tile(np.arange(128, dtype=np.float32), (128, 1)).copy(),
        identh=np.eye(128, dtype=f16),
        row0=ro_w[0].astype(f16), row1=ro_w[1].astype(f16),
        row2=ro_w[2].astype(f16), row3=ro_w[3].astype(f16),
        amap=amap,
    )
    if flags["ln_affine"]:
        shared["lnsc"] = np.repeat(ln_scale.reshape(L, 1, D), 128, axis=1).copy()
        shared["lnbs"] = np.repeat(ln_bias.reshape(L, 1, D), 128, axis=1).copy()
    if flags["wl_b"]:
        shared["wlb"] = np.repeat(wl_b.reshape(L, 1, D), 128, axis=1).astype(f16).copy()
    if flags["conv_b"]:
        shared["convb"] = np.repeat(conv_b.reshape(L, 1, D), 128, axis=1).copy()
    if flags["ro_b"]:
        for i, bb in enumerate(ro_b):
            shared[f"rob{i}"] = np.repeat(bb.reshape(1, -1), 128, axis=0).copy()

    for m in in_maps:
        m.update(shared)
    return in_maps, tiles_b, na_tiles, T, T4, flags


# ----------------------------------------------------------------------------
# device program
# ----------------------------------------------------------------------------

def _build(tiles_b, na_tiles, T, T4, flags):
    nc = bacc.Bacc("TRN2", target_bir_lowering=False, debug=False, num_devices=NC)

    # const AP for activation float biases (Sqrt eps)
    _eps_t = nc.alloc_sbuf_tensor("const-float32-lneps", [128, 1], F32)
    nc.gpsimd.memset(_eps_t.ap(), LN_EPS)
    nc.const_aps.aps[(F32, LN_EPS)] = _eps_t.ap()
    nc.all_engine_barrier()

    TT = T4 * 4
    esrc = nc.dram_tensor("esrc", [128, TT], I32, kind="ExternalInput")
    emeta = nc.dram_tensor("emeta", [128, 2 * TT], F32, kind="ExternalInput")
    batchl = nc.dram_tensor("batchl", [128, NBLK], F32, kind="ExternalInput")
    xid = nc.dram_tensor("xid", [128, NBLK], I32, kind="ExternalInput")
    zrow = nc.dram_tensor("zrow", [128, L], I32, kind="ExternalInput")
    ztgt = nc.dram_tensor("ztgt", [8, 1], I32, kind="ExternalInput")
    xrow = nc.dram_tensor("xrow", [8, 1], I32, kind="ExternalInput")
    amap = nc.dram_tensor("amap", [128, 4], I32, kind="ExternalInput")
    wlw = nc.dram_tensor("wlw", [L, 128, D], F16, kind="ExternalInput")
    convw = nc.dram_tensor("convw", [L, D, D], F16, kind="ExternalInput")
    nemb = nc.dram_tensor("nemb", [NTYPES, D], F32, kind="ExternalInput")
    colidx = nc.dram_tensor("colidx", [128, 128], F32, kind="ExternalInput")
    identh = nc.dram_tensor("identh", [128, 128], F16, kind="ExternalInput")
    row0 = nc.dram_tensor("row0", [6 * D, 768], F16, kind="ExternalInput")
    row1 = nc.dram_tensor("row1", [768, 384], F16, kind="ExternalInput")
    row2 = nc.dram_tensor("row2", [384, 192], F16, kind="ExternalInput")
    row3 = nc.dram_tensor("row3", [192, 1], F16, kind="ExternalInput")
    lnsc = lnbs = wlb = convb = None
    if flags["ln_affine"]:
        lnsc = nc.dram_tensor("lnsc", [L, 128, D], F32, kind="ExternalInput")
        lnbs = nc.dram_tensor("lnbs", [L, 128, D], F32, kind="ExternalInput")
    if flags["wl_b"]:
        wlb = nc.dram_tensor("wlb", [L, 128, D], F16, kind="ExternalInput")
    if flags["conv_b"]:
        convb = nc.dram_tensor("convb", [L, 128, D], F32, kind="ExternalInput")
    robs = None
    if flags["ro_b"]:
        robs = [
            nc.dram_tensor(f"rob{i}", [128, n], F32, kind="ExternalInput")
            for i, n in enumerate([768, 384, 192, 1])
        ]

    out = nc.dram_tensor("out", [B, 1], F32, kind="ExternalOutput")

    tstart = np.concatenate([[0], np.cumsum(tiles_b)]).astype(int)

    with tile.TileContext(nc) as tc:
        with (
            tc.tile_pool(name="dram", bufs=1, space="DRAM") as dram,
            tc.tile_pool(name="consts", bufs=1) as cpool,
            tc.tile_pool(name="lweights", bufs=2) as wpool,
        ):
            y_cs = [dram.tile([NSHARD, D], F16, tag=f"y_c{i}", name=f"y_c{i}")
                    for i in range(L)]
            y_fullAs = [
                dram.tile([GA_ROWS, D], F16, tag=f"y_fullA_{i}",
                          name=f"y_fullA_{i}", addr_space="Shared")
                for i in range(L)
            ]
            y_fullBs = [
                dram.tile([NC * NSLOT_B, D], F16, tag=f"y_fullB_{i}",
                          name=f"y_fullB_{i}", addr_space="Shared")
                for i in range(L)
            ]
            z_loc = dram.tile([ZROWS_L, D], F16, tag="z_loc")
            xchg_in = dram.tile([L, D], F16, tag="xchg_in")
            xchg_all = dram.tile([NC * L, D], F16, tag="xchg_all",
                                 name="xchg_all", addr_space="Shared")
            out_mine = dram.tile([128, 1], F32, tag="out_mine")
            out_all = dram.tile([NC * 128, 1], F32, tag="out_all",
                                name="out_all", addr_space="Shared")

            colidx_t = cpool.tile([128, 128], F32, tag="colidx")
            nc.sync.dma_start(out=colidx_t[:], in_=colidx[:])
            ident_t = cpool.tile([128, 128], F16, tag="identh")
            nc.sync.dma_start(out=ident_t[:], in_=identh[:])
            esrc_sb = cpool.tile([128, TT], I32, tag="esrc_sb")
            nc.sync.dma_start(out=esrc_sb[:], in_=esrc[:])
            emeta_sb = cpool.tile([128, 2 * TT], F32, tag="emeta_sb")
            nc.sync.dma_start(out=emeta_sb[:], in_=emeta[:])
            batchl_sb = cpool.tile([128, NBLK], F32, tag="batchl_sb")
            nc.sync.dma_start(out=batchl_sb[:], in_=batchl[:])
            xid_sb = cpool.tile([128, NBLK], I32, tag="xid_sb")
            nc.sync.dma_start(out=xid_sb[:], in_=xid[:])
            zrow_sb = cpool.tile([128, L], I32, tag="zrow_sb")
            nc.sync.dma_start(out=zrow_sb[:], in_=zrow[:])
            ztgt_sb = cpool.tile([8, 1], I32, tag="ztgt_sb")
            nc.sync.dma_start(out=ztgt_sb[:], in_=ztgt[:])
            xrow_sb = cpool.tile([8, 1], I32, tag="xrow_sb")
            nc.sync.dma_start(out=xrow_sb[:], in_=xrow[:])
            amap_sb = cpool.tile([128, 4], I32, tag="amap_sb")
            nc.sync.dma_start(out=amap_sb[:], in_=amap[:])

            # persistent local y (residual input), one 128x256 slice per block
            y_sb = cpool.tile([128, NBLK, D], F16, tag="y_sb")

            # pooling indicators (layer-invariant)
            ind_ts = []
            for b in range(NBLK):
                ind_b = cpool.tile([128, 128], F16, tag=f"ind{b}", name=f"ind{b}")
                nc.vector.tensor_tensor(
                    out=ind_b[:],
                    in0=batchl_sb[:, b:b + 1].to_broadcast([128, 128]),
                    in1=colidx_t[:],
                    op=ALU.is_equal,
                )
                ind_ts.append(ind_b)
            # segment indicators (layer-invariant)
            st_ts = []
            for t in range(T):
                st_t = cpool.tile([128, 128], F16, tag=f"st{t}", name=f"st{t}")
                nc.vector.tensor_tensor(
                    out=st_t[:],
                    in0=emeta_sb[:, t:t + 1].to_broadcast([128, 128]),
                    in1=colidx_t[:],
                    op=ALU.is_equal,
                )
                st_ts.append(st_t)

            # zero z_loc (NaN hygiene for unwritten rows)
            with tc.tile_pool(name="zz", bufs=1) as zz:
                zt0 = zz.tile([128, D], F16)
                nc.vector.memset(zt0[:], 0.0)
                for k in range(4):
                    nc.sync.dma_start(
                        out=z_loc[k * 128:(k + 1) * 128, :], in_=zt0[:])
                nc.sync.dma_start(out=z_loc[512:ZROWS_L, :],
                                  in_=zt0[:ZROWS_L - 512, :])

            def ln_block(lp, src_tile, hsum, b, li_next, use_reduce=False):
                """Finalize LN stats and write y for block b of layer li_next.

                hsum must hold sum(src) (f32 [128,1]); computes sumsq via
                Square activation accum, then y=(src-mu)*rsqrt(var+eps).
                """
                sq = lp.tile([128, D], F16, tag="sq")
                ssum = lp.tile([128, 1], F32, tag="ssum")
                nc.scalar.activation(sq[:], src_tile, ACTF.Square,
                                     accum_out=ssum[:])
                mu = lp.tile([128, 1], F32, tag="mu")
                nc.vector.tensor_scalar(
                    out=mu[:], in0=hsum, scalar1=1.0 / D, scalar2=None,
                    op0=ALU.mult)
                d1 = lp.tile([128, 1], F32, tag="d1")
                nc.vector.tensor_tensor(out=d1[:], in0=hsum, in1=hsum,
                                        op=ALU.mult)
                d2 = lp.tile([128, 1], F32, tag="d2")
                nc.vector.tensor_scalar(
                    out=d2[:], in0=ssum[:], scalar1=float(D), scalar2=d1[:],
                    op0=ALU.mult, op1=ALU.subtract)
                sd = lp.tile([128, 1], F32, tag="sd")
                nc.scalar.activation(sd[:], d2[:], ACTF.Sqrt, bias=LN_EPS,
                                     scale=1.0 / (D * D))
                rs = lp.tile([128, 1], F32, tag="rs")
                nc.vector.reciprocal(rs[:], sd[:])
                nc.vector.tensor_scalar(
                    out=y_sb[:, b, :], in0=src_tile, scalar1=mu[:],
                    scalar2=rs[:], op0=ALU.subtract, op1=ALU.mult)
                if flags["ln_affine"]:
                    nc.vector.tensor_tensor(
                        out=y_sb[:, b, :], in0=y_sb[:, b, :],
                        in1=lnsc_t[:], op=ALU.mult)
                    nc.vector.tensor_tensor(
                        out=y_sb[:, b, :], in0=y_sb[:, b, :],
                        in1=lnbs_t[:], op=ALU.add)
                nc.sync.dma_start(
                    out=y_cs[li_next][b * 128:(b + 1) * 128, :],
                    in_=y_sb[:, b, :])

            def ag_chunk(li, which):
                y_c = y_cs[li]
                if which == 0:
                    nc.gpsimd.collective_compute(
                        "AllGather", ALU.bypass,
                        replica_groups=[list(range(NC))],
                        ins=[y_c[0:NSLOT_A, :].opt()],
                        outs=[y_fullAs[li][:].opt()],
                    )
                else:
                    nc.gpsimd.collective_compute(
                        "AllGather", ALU.bypass,
                        replica_groups=[list(range(NC))],
                        ins=[y_c[NSLOT_A:, :].opt()],
                        outs=[y_fullBs[li][:].opt()],
                    )

            # ---------- embedding + LN of layer-0 input ----------
            with tc.tile_pool(name="emb", bufs=4) as ep:
                for b in range(NBLK):
                    et = ep.tile([128, D], F32, tag="et")
                    nc.gpsimd.indirect_dma_start(
                        out=et[:], out_offset=None, in_=nemb[:],
                        in_offset=bass.IndirectOffsetOnAxis(
                            ap=xid_sb[:, b:b + 1], axis=0),
                    )
                    hsum = ep.tile([128, 1], F32, tag="hsum")
                    nc.vector.tensor_reduce(
                        out=hsum[:], in_=et[:], axis=mybir.AxisListType.X,
                        op=ALU.add)
                    ln_block(ep, et[:], hsum[:], b, 0)
                    if b == NA - 1:
                        ag_chunk(0, 0)
            ag_chunk(0, 1)

            # ---------- layers ----------
            with (
                tc.tile_pool(name="lp", bufs=3) as lp,
                tc.tile_pool(name="edge", bufs=6) as xp,
                tc.tile_pool(name="blk", bufs=3) as bp,
                tc.tile_pool(name="ps_nd", bufs=3, space="PSUM") as ps_nd,
                tc.tile_pool(name="ps_xt", bufs=2, space="PSUM") as ps_xt,
                tc.tile_pool(name="ps_h", bufs=1, space="PSUM") as ps_h,
                tc.tile_pool(name="ps_pool", bufs=1, space="PSUM") as ps_pool,
            ):
                for li in range(L):
                    y_fullA, y_fullB = y_fullAs[li], y_fullBs[li]
                    wlw_t = wpool.tile([128, D], F16, tag="wlw")
                    nc.sync.dma_start(out=wlw_t[:], in_=wlw[li])
                    cw0 = wpool.tile([128, D], F16, tag="cw0")
                    nc.sync.dma_start(out=cw0[:], in_=convw[li, 0:128, :])
                    cw1 = wpool.tile([128, D], F16, tag="cw1")
                    nc.sync.dma_start(out=cw1[:], in_=convw[li, 128:256, :])
                    if flags["ln_affine"]:
                        lnsc_t = wpool.tile([128, D], F32, tag="lnsc")
                        nc.sync.dma_start(out=lnsc_t[:], in_=lnsc[li])
                        lnbs_t = wpool.tile([128, D], F32, tag="lnbs")
                        nc.sync.dma_start(out=lnbs_t[:], in_=lnbs[li])
                    if flags["wl_b"]:
                        wlb_t = wpool.tile([128, D], F16, tag="wlb")
                        nc.sync.dma_start(out=wlb_t[:], in_=wlb[li])
                    if flags["conv_b"]:
                        convb_t = wpool.tile([128, D], F32, tag="convb")
                        nc.sync.dma_start(out=convb_t[:], in_=convb[li])

                    ppool = ps_pool.tile([128, D], F32, tag="ppool")
                    for b in range(NBLK):
                        nd = ps_nd.tile([128, 2 * D], F32, tag="nd")
                        t0, t1 = int(tstart[b]), int(tstart[b + 1])
                        for t in range(t0, t1):
                            ysrc = xp.tile([128, D], F16, tag="ysrc")
                            src_ap = y_fullA[:] if t - t0 < int(na_tiles[b]) \
                                else y_fullB[:]
                            nc.gpsimd.indirect_dma_start(
                                out=ysrc[:], out_offset=None, in_=src_ap,
                                in_offset=bass.IndirectOffsetOnAxis(
                                    ap=esrc_sb[:, t:t + 1], axis=0),
                            )
                            pre = xp.tile([128, D], F16, tag="pre")
                            nc.vector.scalar_tensor_tensor(
                                out=pre[:], in0=wlw_t[:],
                                scalar=emeta_sb[:, TT + t:TT + t + 1],
                                in1=ysrc[:], op0=ALU.mult, op1=ALU.add,
                            )
                            if flags["wl_b"]:
                                nc.vector.tensor_tensor(
                                    out=pre[:], in0=pre[:], in1=wlb_t[:],
                                    op=ALU.add)
                            msg = xp.tile([128, D], F16, tag="msg")
                            nc.scalar.activation(msg[:], pre[:], ACTF.Relu)
                            ev = xp.tile([128, 2 * D], F16, tag="ev")
                            nc.scalar.activation(ev[:, :D], msg[:], ACTF.Exp)
                            nc.vector.tensor_tensor(
                                out=ev[:, D:], in0=msg[:], in1=ev[:, :D],
                                op=ALU.mult)
                            nc.tensor.matmul(
                                out=nd[:], lhsT=st_ts[t][:], rhs=ev[:],
                                start=(t == t0), stop=(t == t1 - 1),
                            )
                        # block post: softmax-agg + residual + conv + relu
                        td = bp.tile([128, D], F32, tag="td")
                        nc.vector.tensor_scalar_max(td[:], nd[:, :D], 1e-16)
                        rec = bp.tile([128, D], F32, tag="rec")
                        nc.vector.reciprocal_approx_fast(out=rec[:], in_=td[:])
                        xv = bp.tile([128, D], F16, tag="xv")
                        nc.vector.tensor_tensor(
                            out=xv[:], in0=nd[:, D:], in1=rec[:], op=ALU.mult)
                        nc.vector.tensor_tensor(
                            out=xv[:], in0=xv[:], in1=y_sb[:, b, :], op=ALU.add)
                        pxt = ps_xt.tile([128, D], F16, tag="pxt")
                        nc.tensor.transpose(
                            out=pxt[:, 0:128], in_=xv[:, 0:128],
                            identity=ident_t[:])
                        nc.tensor.transpose(
                            out=pxt[:, 128:256], in_=xv[:, 128:256],
                            identity=ident_t[:])
                        xts = bp.tile([128, D], F16, tag="xts")
                        nc.vector.tensor_copy(out=xts[:], in_=pxt[:])
                        ph = ps_h.tile([128, D], F32, tag="ph")
                        nc.tensor.matmul(
                            out=ph[:], lhsT=xts[:, 0:128], rhs=cw0[:],
                            start=True, stop=False)
                        nc.tensor.matmul(
                            out=ph[:], lhsT=xts[:, 128:256], rhs=cw1[:],
                            start=False, stop=True)
                        if flags["conv_b"]:
                            nc.vector.tensor_tensor(
                                out=ph[:], in0=ph[:], in1=convb_t[:],
                                op=ALU.add)
                        hb = bp.tile([128, D], F16, tag="hb")
                        if li < L - 1:
                            hsum = bp.tile([128, 1], F32, tag="hsum")
                            nc.scalar.activation(hb[:], ph[:], ACTF.Relu,
                                                 accum_out=hsum[:])
                            ln_block(bp, hb[:], hsum[:], b, li + 1)
                            if b == NA - 1:
                                ag_chunk(li + 1, 0)
                        else:
                            nc.scalar.activation(hb[:], ph[:], ACTF.Relu)
                        nc.tensor.matmul(
                            out=ppool[:], lhsT=ind_ts[b][:], rhs=hb[:],
                            start=(b == 0), stop=(b == NBLK - 1),
                        )
                    if li < L - 1:
                        ag_chunk(li + 1, 1)
                    # pool epilogue: scatter z rows + boundary-exchange row
                    zp = bp.tile([128, D], F16, tag="zp")
                    nc.vector.tensor_copy(out=zp[:], in_=ppool[:])
                    nc.gpsimd.indirect_dma_start(
                        out=z_loc[:],
                        out_offset=bass.IndirectOffsetOnAxis(
                            ap=zrow_sb[:, li:li + 1], axis=0),
                        in_=zp[:], in_offset=None,
                    )
                    nc.sync.dma_start(out=xchg_in[li:li + 1, :], in_=zp[0:1, :])

            # ---------- boundary exchange + readout ----------
            nc.gpsimd.collective_compute(
                "AllGather", ALU.bypass,
                replica_groups=[list(range(NC))],
                ins=[xchg_in[:].opt()], outs=[xchg_all[:].opt()],
            )
            with (
                tc.tile_pool(name="row", bufs=1) as rw,
                tc.tile_pool(name="ro", bufs=2) as ro,
                tc.tile_pool(name="ps_a", bufs=1, space="PSUM") as psa,
                tc.tile_pool(name="ps_b", bufs=1, space="PSUM") as psb,
                tc.tile_pool(name="ps_t", bufs=2, space="PSUM") as pst,
                tc.tile_pool(name="ps_o", bufs=1, space="PSUM") as pso,
            ):
                # add boundary partials from the next core into owned z rows
                xg = ro.tile([8, D], F16, tag="xg")
                nc.gpsimd.indirect_dma_start(
                    out=xg[:], out_offset=None, in_=xchg_all[:],
                    in_offset=bass.IndirectOffsetOnAxis(ap=xrow_sb[:], axis=0))
                zg = ro.tile([8, D], F16, tag="zg")
                nc.gpsimd.indirect_dma_start(
                    out=zg[:], out_offset=None, in_=z_loc[:],
                    in_offset=bass.IndirectOffsetOnAxis(ap=ztgt_sb[:], axis=0))
                nc.vector.tensor_tensor(out=zg[:], in0=zg[:], in1=xg[:],
                                        op=ALU.add)
                nc.gpsimd.indirect_dma_start(
                    out=z_loc[:],
                    out_offset=bass.IndirectOffsetOnAxis(ap=ztgt_sb[:], axis=0),
                    in_=zg[:], in_offset=None,
                )

                w0t = []
                for f in range(12):
                    w = rw.tile([128, 768], F16, tag=f"w0_{f}")
                    nc.sync.dma_start(out=w[:], in_=row0[f * 128:(f + 1) * 128, :])
                    w0t.append(w)
                w1t = []
                for f in range(6):
                    w = rw.tile([128, 384], F16, tag=f"w1_{f}")
                    nc.sync.dma_start(out=w[:], in_=row1[f * 128:(f + 1) * 128, :])
                    w1t.append(w)
                w2t = []
                for f in range(3):
                    w = rw.tile([128, 192], F16, tag=f"w2_{f}")
                    nc.sync.dma_start(out=w[:], in_=row2[f * 128:(f + 1) * 128, :])
                    w2t.append(w)
                w3a = rw.tile([128, 1], F16, tag="w3a")
                nc.sync.dma_start(out=w3a[:], in_=row3[0:128, :])
                w3b = rw.tile([64, 1], F16, tag="w3b")
                nc.sync.dma_start(out=w3b[:], in_=row3[128:192, :])
                robt = []
                if flags["ro_b"]:
                    for i, n in enumerate([768, 384, 192, 1]):
                        w = rw.tile([128, n], F32, tag=f"rob{i}")
                        nc.sync.dma_start(out=w[:], in_=robs[i][:])
                        robt.append(w)

                # z for owned graphs: [ZG, L*D] contiguous read
                zt_all = rw.tile([128, L * D], F16, tag="zt_all")
                nc.vector.memset(zt_all[:], 0.0)
                nc.sync.dma_start(
                    out=zt_all[:ZG, :],
                    in_=z_loc[0:ZG * L, :].rearrange("(q l) d -> q (l d)", l=L),
                )

                def transpose_chunk(src_ap, kdim):
                    pt = pst.tile([128, 128], F16, tag="pt")
                    nc.tensor.transpose(
                        out=pt[:kdim, :], in_=src_ap, identity=ident_t[:])
                    ct = ro.tile([128, 128], F16, tag="ct")
                    nc.vector.tensor_copy(out=ct[:kdim, :], in_=pt[:kdim, :])
                    return ct

                pA = psa.tile([128, 512], F32, tag="pA")
                pB = psb.tile([128, 256], F32, tag="pB")
                for f in range(12):
                    zt = transpose_chunk(zt_all[:, 128 * f:128 * (f + 1)], 128)
                    nc.tensor.matmul(
                        out=pA[:], lhsT=zt[:], rhs=w0t[f][:, 0:512],
                        start=(f == 0), stop=(f == 11))
                    nc.tensor.matmul(
                        out=pB[:], lhsT=zt[:], rhs=w0t[f][:, 512:768],
                        start=(f == 0), stop=(f == 11))
                z1 = ro.tile([128, 768], F16, tag="z1")
                if flags["ro_b"]:
                    nc.vector.tensor_tensor(
                        out=pA[:], in0=pA[:], in1=robt[0][:, 0:512], op=ALU.add)
                    nc.vector.tensor_tensor(
                        out=pB[:], in0=pB[:], in1=robt[0][:, 512:768], op=ALU.add)
                nc.scalar.activation(z1[:, 0:512], pA[:], ACTF.Gelu)
                nc.scalar.activation(z1[:, 512:768], pB[:], ACTF.Gelu)

                p2 = psa.tile([128, 384], F32, tag="p2")
                for f in range(6):
                    zt = transpose_chunk(z1[:, 128 * f:128 * (f + 1)], 128)
                    nc.tensor.matmul(
                        out=p2[:], lhsT=zt[:], rhs=w1t[f][:],
                        start=(f == 0), stop=(f == 5))
                if flags["ro_b"]:
                    nc.vector.tensor_tensor(
                        out=p2[:], in0=p2[:], in1=robt[1][:], op=ALU.add)
                z2 = ro.tile([128, 384], F16, tag="z2")
                nc.scalar.activation(z2[:], p2[:], ACTF.Gelu)

                p3 = psb.tile([128, 192], F32, tag="p3")
                for f in range(3):
                    zt = transpose_chunk(z2[:, 128 * f:128 * (f + 1)], 128)
                    nc.tensor.matmul(
                        out=p3[:], lhsT=zt[:], rhs=w2t[f][:],
                        start=(f == 0), stop=(f == 2))
                if flags["ro_b"]:
                    nc.vector.tensor_tensor(
                        out=p3[:], in0=p3[:], in1=robt[2][:], op=ALU.add)
                z3 = ro.tile([128, 192], F16, tag="z3")
                nc.scalar.activation(z3[:], p3[:], ACTF.Gelu)

                po = pso.tile([128, 1], F32, tag="po")
                zt = transpose_chunk(z3[:, 0:128], 128)
                nc.tensor.matmul(out=po[:], lhsT=zt[:], rhs=w3a[:],
                                 start=True, stop=False)
                zt = transpose_chunk(z3[:, 128:192], 64)
                nc.tensor.matmul(out=po[:], lhsT=zt[:64, :], rhs=w3b[:],
                                 start=False, stop=True)
                oc = ro.tile([128, 1], F32, tag="oc")
                if flags["ro_b"]:
                    nc.vector.tensor_tensor(
                        out=po[:], in0=po[:], in1=robt[3][:], op=ALU.add)
                nc.vector.tensor_copy(out=oc[:], in_=po[:])
                nc.sync.dma_start(out=out_mine[:], in_=oc[:])

                # gather per-core outputs and assemble [512,1]
                nc.gpsimd.collective_compute(
                    "AllGather", ALU.bypass,
                    replica_groups=[list(range(NC))],
                    ins=[out_mine[:].opt()], outs=[out_all[:].opt()],
                )
                og = ro.tile([128, 4], F32, tag="og")
                for j in range(4):
                    nc.gpsimd.indirect_dma_start(
                        out=og[:, j:j + 1], out_offset=None, in_=out_all[:],
                        in_offset=bass.IndirectOffsetOnAxis(
                            ap=amap_sb[:, j:j + 1], axis=0),
                    )
                for j in range(4):
                    nc.sync.dma_start(
                        out=out[128 * j:128 * (j + 1), :], in_=og[:, j:j + 1])

    nc.compile()
    return nc


# ----------------------------------------------------------------------------
# entry point
# ----------------------------------------------------------------------------

def kernel(**inputs):
    in_maps, tiles_b, na_tiles, T, T4, flags = _prep(inputs)
    key = (tuple(tiles_b.tolist()), tuple(na_tiles.tolist()),
           tuple(sorted(flags.items())))
    if key not in _prog_cache:
        _prog_cache[key] = _build(tiles_b, na_tiles, T, T4, flags)
    nc = _prog_cache[key]

    kwargs = {}
    if TRACE:
        kwargs = dict(trace=True, trace_cores=TRACE_CORES)
    res = run_bass_kernel_spmd(nc, in_maps, list(range(NC)), **kwargs)
    LAST_RESULT["exec_time_ns"] = getattr(res, "exec_time_ns", None)
    LAST_RESULT["res"] = res
    return np.asarray(res.results[0]["out"], dtype=np.float32)
